# revision 10
# baseline (speedup 1.0000x reference)
"""GCN encoder (2x GCNConv + BatchNorm + ReLU) on 8 Trainium2 NeuronCores.

Strategy (graph/data parallel, per sharding hint):
- Nodes are permuted (degree-sorted, round-robin dealt) and sharded across the
  8 cores; each core owns 49 "windows" of 128 destination nodes.
- norm factorizes: norm(s,d) = dis[s]*dis[d].  Source scaling dis[s] is folded
  into the feature tables (h~ = dis * h); destination scaling dis[d] is applied
  on PSUM eviction.  Messages then aggregate with a *constant identity* matmul:
  for each window, gathered source rows land in "slots" (slot = local dst id),
  and chunk matmuls with a preloaded identity accumulate them in PSUM.
- Gathers use the int16 dma_gather embedding path.  int16 limits addressing to
  32768 rows, so the feature table is split in two halves (cores 0-3 / 4-7) and
  each window runs one gather per half; pad slots point at an all-zero row.
- h1 = x @ W1 is computed redundantly on every core (full table in local HBM).
  h2's input depends on BN1 (global stats -> AllReduce); each core computes its
  own shard of h2 = relu(bn(conv1)) @ W2 and an AllGather replicates the table.
- BatchNorm uses E[x^2]-mean^2 with sums computed by ones-vector matmuls
  (partition reduction) accumulated in PSUM across windows, then AllReduce.
- b1/b2 are ignored: a per-feature constant added before BatchNorm cancels
  exactly in (x - mean).
"""

import sys

sys.path.insert(0, "/opt/trn_rl_repo")

import numpy as np

N_CORES = 8
P = 128
EPS = 1e-5

_FULL_CFG = dict(N=50000, IN=512, D1=256, D2=128)


# ---------------------------------------------------------------- host preprocessing

def _preprocess(edge_index, N):
    """Graph preprocessing: node permutation, slot/chunk assignment, gather
    indices.  Pure integer work on the host."""
    src = np.asarray(edge_index[0], dtype=np.int64)
    dst = np.asarray(edge_index[1], dtype=np.int64)
    # append self loops
    loop = np.arange(N, dtype=np.int64)
    S = np.concatenate([src, loop])
    D = np.concatenate([dst, loop])

    deg = np.bincount(D, minlength=N)  # >= 1 (self loop)
    dis = (1.0 / np.sqrt(deg.astype(np.float64))).astype(np.float32)

    real_pc = N // N_CORES
    WPC = (real_pc + P - 1) // P          # windows per core
    SLOTS = WPC * P                        # slot positions per core
    BLK = SLOTS + 1                        # +1 trailing zero row per core block

    # deal nodes to cores round-robin in degree-desc order
    order = np.argsort(-deg, kind="stable")
    core_of = np.empty(N, dtype=np.int64)
    core_of[order] = np.arange(N) % N_CORES

    half_node = core_of >= (N_CORES // 2)  # False = lo table half
    halfE = half_node[S]

    deg_lo = np.bincount(D[~halfE], minlength=N)
    deg_hi = deg - deg_lo

    # position of each node within its core: sort by (deg_lo desc, deg_hi desc),
    # then re-sort blocks of 768 by deg_hi — keeps per-window max(deg_lo) and
    # max(deg_hi) both tight (gather padding ~18% instead of ~31%)
    pos = np.empty(N, dtype=np.int64)
    node_by_cp = np.full((N_CORES, SLOTS), -1, dtype=np.int64)
    RESORT_BLK = 768
    for c in range(N_CORES):
        nodes_c = np.flatnonzero(core_of == c)
        o = np.lexsort((-deg_hi[nodes_c], -deg_lo[nodes_c]))
        for s in range(0, len(o), RESORT_BLK):
            blk = o[s : s + RESORT_BLK]
            o[s : s + RESORT_BLK] = blk[np.argsort(-deg_hi[nodes_c][blk], kind="stable")]
        snodes = nodes_c[o]
        pos[snodes] = np.arange(len(snodes))
        node_by_cp[c, : len(snodes)] = snodes

    # per-core per-window chunk counts -> global max (SPMD static shapes)
    dlo_cp = np.zeros((N_CORES, SLOTS), dtype=np.int64)
    dhi_cp = np.zeros((N_CORES, SLOTS), dtype=np.int64)
    m = node_by_cp >= 0
    dlo_cp[m] = deg_lo[node_by_cp[m]]
    dhi_cp[m] = deg_hi[node_by_cp[m]]
    NLc = dlo_cp.reshape(N_CORES, WPC, P).max(axis=2)
    NHc = dhi_cp.reshape(N_CORES, WPC, P).max(axis=2)
    NL = NLc.max(axis=0)
    NH = NHc.max(axis=0)

    # idx segment offsets: per window [lo seg][hi seg], chunk-major inside
    seg = (NL + NH) * P
    base = np.concatenate([[0], np.cumsum(seg)])
    offL = base[:-1]
    offH = base[:-1] + NL * P
    TOT = int(base[-1])

    # chunk index of each edge among its (dst, half) group
    key = D * 2 + halfE
    ksort = np.argsort(key, kind="stable")
    skey = key[ksort]
    starts = np.concatenate([[0], np.flatnonzero(np.diff(skey)) + 1])
    group_len = np.diff(np.concatenate([starts, [len(skey)]]))
    chunk_sorted = np.arange(len(skey)) - np.repeat(starts, group_len)
    chunk = np.empty(len(S), dtype=np.int64)
    chunk[ksort] = chunk_sorted

    cD = core_of[D]
    wD = pos[D] // P
    slotD = pos[D] % P
    absrow = core_of[S] * BLK + pos[S]
    rel = np.where(~halfE, absrow, absrow - (N_CORES // 2) * BLK)
    assert rel.max() < 32768
    epos = np.where(~halfE, offL[wD], offH[wD]) + chunk * P + slotD

    PADIDX = SLOTS  # each block's trailing zero row (rel within half view)
    flat = np.full(N_CORES * TOT, PADIDX, dtype=np.int16)
    flat[cD * TOT + epos] = rel.astype(np.int16)
    flat = flat.reshape(N_CORES, TOT)
    # wrap: idx i -> [i%16, i//16], replicated across the 8 groups of 16 rows
    wrapped16 = flat.reshape(N_CORES, TOT // 16, 16).transpose(0, 2, 1)
    idx_wrapped = np.tile(wrapped16, (1, P // 16, 1))  # [cores, 128, TOT/16]

    # per-core dis (by slot), 1.0 for dummies
    dis_cp = np.ones((N_CORES, SLOTS), dtype=np.float32)
    dis_cp[m] = dis[node_by_cp[m]]
    dismy = dis_cp.reshape(N_CORES, WPC, P).transpose(0, 2, 1)  # [c, 128, WPC]

    # global dis in table-data-row order (for phase 1), [128, NTILES]
    NTILES = N_CORES * WPC
    disall = dis_cp.reshape(NTILES, P).T.copy()  # [128, NTILES]

    # stats mask: last window has (SLOTS - real_pc) dummy rows at the end
    n_dummy = SLOTS - real_pc
    statmask = np.ones((P, 2), dtype=np.float32)
    if n_dummy:
        statmask[P - n_dummy :, 1] = 0.0

    waste = float(seg.sum()) / max(1, len(S) / N_CORES) - 1.0
    return dict(
        WPC=WPC, SLOTS=SLOTS, BLK=BLK, NTILES=NTILES,
        NL=NL.astype(int), NH=NH.astype(int), TOT=TOT,
        offL=offL, offH=offH,
        idx_wrapped=idx_wrapped, dismy=dismy, disall=disall,
        statmask=statmask, node_by_cp=node_by_cp, pos=pos, core_of=core_of,
        dis=dis, real_pc=real_pc, waste=waste,
    )


def _pack_inputs(x, W1, W2, pp, cfg):
    """Build the per-core / shared device input arrays."""
    import ml_dtypes

    bf16 = ml_dtypes.bfloat16
    N, IN, D1, D2 = cfg["N"], cfg["IN"], cfg["D1"], cfg["D2"]
    WPC, SLOTS, NTILES = pp["WPC"], pp["SLOTS"], pp["NTILES"]
    KC = IN // P

    # x permuted to table order, zeros for dummies -> [NTILES, 128(p=k), KC, 128(j=row)]
    xperm = np.zeros((N_CORES * SLOTS, IN), dtype=np.float32)
    m = pp["node_by_cp"] >= 0
    xperm[m.reshape(-1)] = x[pp["node_by_cp"][m]]
    xb = (
        xperm.reshape(NTILES, P, KC, P)   # [b, j, kc, p]
        .transpose(0, 3, 2, 1)            # [b, p, kc, j]
        .astype(bf16)
    )
    w1b = W1.reshape(KC, P, D1).transpose(1, 0, 2).astype(bf16)   # [p, kc, D1]
    w2b = W2.reshape(D1 // P, P, D2).transpose(1, 0, 2).astype(bf16)  # [p, kc, D2]
    return xb, w1b, w2b


# ---------------------------------------------------------------- device kernel

def _build_kernel(cfg, pp, phases=5):
    import concourse.bacc as bacc
    import concourse.mybir as mybir
    import concourse.tile as tile
    from concourse.masks import make_identity
    from contextlib import ExitStack

    N, IN, D1, D2 = cfg["N"], cfg["IN"], cfg["D1"], cfg["D2"]
    WPC, SLOTS, BLK, NTILES = pp["WPC"], pp["SLOTS"], pp["BLK"], pp["NTILES"]
    NL, NH, TOT = pp["NL"], pp["NH"], pp["TOT"]
    offL, offH = pp["offL"], pp["offH"]
    KC = IN // P
    KC2 = D1 // P
    HB = (N_CORES // 2) * BLK        # hi half base row
    NROWS = N_CORES * BLK            # table rows
    NTmax = int((NL + NH).max())
    RG = [list(range(N_CORES))]
    f32, bf16, i16 = mybir.dt.float32, mybir.dt.bfloat16, mybir.dt.int16
    f32r = mybir.dt.float32r
    AF = mybir.ActivationFunctionType

    nc = bacc.Bacc(num_devices=N_CORES)

    # ---- I/O
    xb_d = nc.dram_tensor("xb", [NTILES, P, KC, P], bf16, kind="ExternalInput")
    w1_d = nc.dram_tensor("w1b", [P, KC, D1], bf16, kind="ExternalInput")
    w2_d = nc.dram_tensor("w2b", [P, KC2, D2], bf16, kind="ExternalInput")
    idx_d = nc.dram_tensor("idx", [P, TOT // 16], i16, kind="ExternalInput")
    disall_d = nc.dram_tensor("disall", [P, NTILES], f32, kind="ExternalInput")
    dismy_d = nc.dram_tensor("dismy", [P, WPC], f32, kind="ExternalInput")
    mask_d = nc.dram_tensor("statmask", [P, 2], f32, kind="ExternalInput")
    g1_d = nc.dram_tensor("gamma1", [1, D1], f32, kind="ExternalInput")
    b1_d = nc.dram_tensor("beta1", [1, D1], f32, kind="ExternalInput")
    g2_d = nc.dram_tensor("gamma2", [1, D2], f32, kind="ExternalInput")
    b2_d = nc.dram_tensor("beta2", [1, D2], f32, kind="ExternalInput")
    out_d = nc.dram_tensor("out", [SLOTS, D2], f32, kind="ExternalOutput")

    # ---- internal DRAM
    h1tab = nc.dram_tensor("h1tab", [NROWS, D1], bf16, kind="Internal")
    h2shard = nc.dram_tensor("h2shard", [BLK, D2], bf16, kind="Internal")
    h2tab = nc.dram_tensor("h2tab", [NROWS, D2], bf16, kind="Internal", addr_space="Shared")
    ar1_in = nc.dram_tensor("ar1_in", [1, 2 * D1], f32, kind="Internal")
    ar1_out = nc.dram_tensor("ar1_out", [1, 2 * D1], f32, kind="Internal", addr_space="Shared")
    ar2_in = nc.dram_tensor("ar2_in", [1, 2 * D2], f32, kind="Internal")
    ar2_out = nc.dram_tensor("ar2_out", [1, 2 * D2], f32, kind="Internal", addr_space="Shared")

    import concourse.bass as bass

    def pad_rows_ap(tensor, D):
        # rows {c*BLK + SLOTS : c in 0..7} of a [NROWS, D] table
        return bass.AP(tensor, SLOTS * D, [[BLK * D, N_CORES], [1, D]])

    with tile.TileContext(nc) as tc:
        es = ExitStack()
        with es:
            cpool = es.enter_context(tc.tile_pool(name="const", bufs=1))
            ident_b = cpool.tile([P, P], bf16)
            make_identity(nc, ident_b[:])
            ident_f = cpool.tile([P, P], f32)
            make_identity(nc, ident_f[:])
            w1_s = cpool.tile([P, KC, D1], bf16)
            nc.sync.dma_start(out=w1_s[:], in_=w1_d[:, :, :])
            w2_s = cpool.tile([P, KC2, D2], bf16)
            nc.sync.dma_start(out=w2_s[:], in_=w2_d[:, :, :])
            disall_s = cpool.tile([P, NTILES], f32)
            nc.sync.dma_start(out=disall_s[:], in_=disall_d[:, :])
            dismy_s = cpool.tile([P, WPC], f32)
            nc.sync.dma_start(out=dismy_s[:], in_=dismy_d[:, :])
            mask_s = cpool.tile([P, 2], f32)
            nc.sync.dma_start(out=mask_s[:], in_=mask_d[:, :])
            mask_b = cpool.tile([P, 2], bf16)
            nc.vector.tensor_copy(out=mask_b[:], in_=mask_s[:])
            idx_s = cpool.tile([P, TOT // 16], i16)
            nc.sync.dma_start(out=idx_s[:], in_=idx_d[:, :])
            gb_s = cpool.tile([1, 2 * D1 + 2 * D2], f32)  # gamma1|beta1|gamma2|beta2
            nc.sync.dma_start(out=gb_s[:, 0:D1], in_=g1_d[:, :])
            nc.sync.dma_start(out=gb_s[:, D1 : 2 * D1], in_=b1_d[:, :])
            nc.sync.dma_start(out=gb_s[:, 2 * D1 : 2 * D1 + D2], in_=g2_d[:, :])
            nc.sync.dma_start(out=gb_s[:, 2 * D1 + D2 :], in_=b2_d[:, :])

            # zero pad rows of h1tab (one strided DMA)
            zrow = cpool.tile([N_CORES, D1], bf16)
            nc.vector.memset(zrow[:], 0)
            nc.gpsimd.dma_start(out=pad_rows_ap(h1tab, D1), in_=zrow[:])

            # ---------------- phase 1: h1tab = dis * (x @ W1), all rows ----------------
            with (
                tc.tile_pool(name="p1x", bufs=3) as xpool,
                tc.tile_pool(name="p1s", bufs=2) as spool,
                tc.tile_pool(name="p1p", bufs=2, space="PSUM") as ppool1,
            ):
                for blk in range(N_CORES):
                    stage = spool.tile([P, WPC, D1], bf16, tag="stage")
                    for t in range(WPC):
                        b = blk * WPC + t
                        xt = xpool.tile([P, KC, P], bf16, tag="xt")
                        nc.sync.dma_start(out=xt[:], in_=xb_d[b])
                        ps = ppool1.tile([P, D1], f32, tag="ps1")
                        for kc in range(KC):
                            nc.tensor.matmul(
                                out=ps[:], lhsT=xt[:, kc, :], rhs=w1_s[:, kc, :],
                                start=(kc == 0), stop=(kc == KC - 1),
                            )
                        nc.vector.tensor_scalar_mul(
                            stage[:, t, :], ps[:], disall_s[:, b : b + 1]
                        )
                    nc.gpsimd.dma_start(
                        out=h1tab[blk * BLK : blk * BLK + SLOTS, :].rearrange(
                            "(t p) d -> p t d", p=P
                        ),
                        in_=stage[:],
                    )

            # ---------------- conv1: window aggregation ----------------
            if phases < 2:
                outst = cpool.tile([P, WPC, D2], f32)
                nc.vector.memset(outst[:], 0)
                nc.gpsimd.dma_start(
                    out=out_d[0:SLOTS, :].rearrange("(t p) d -> p t d", p=P),
                    in_=outst[:],
                )
                return nc
            o1_pool = es.enter_context(tc.tile_pool(name="o1", bufs=1))
            o1_all = o1_pool.tile([P, WPC, D1], f32)
            lo_view = h1tab[0:HB, :]
            hi_view = h1tab[HB:NROWS, :]
            with (
                tc.tile_pool(name="g1", bufs=2) as gpool,
                tc.tile_pool(name="sq1", bufs=2) as sqpool,
                tc.tile_pool(name="c1p", bufs=2, space="PSUM") as wpool,
                tc.tile_pool(name="st1p", bufs=1, space="PSUM") as stpool,
            ):
                st_s = stpool.tile([1, D1], f32, tag="st_s")
                st_q = stpool.tile([1, D1], f32, tag="st_q")
                for w in range(WPC):
                    nl, nh = int(NL[w]), int(NH[w])
                    nt = nl + nh
                    gb = gpool.tile([P, NTmax, D1], bf16, tag="g1")
                    if nl:
                        nc.gpsimd.dma_gather(
                            gb[:, 0:nl, :], lo_view, idx_s[:, offL[w] // 16 : offL[w] // 16 + nl * 8],
                            nl * P, nl * P, D1, elem_step=D1, single_packet=False,
                        )
                    if nh:
                        nc.gpsimd.dma_gather(
                            gb[:, nl:nt, :], hi_view, idx_s[:, offH[w] // 16 : offH[w] // 16 + nh * 8],
                            nh * P, nh * P, D1, elem_step=D1, single_packet=False,
                        )
                    ps = wpool.tile([P, D1], f32, tag="win1")
                    for j in range(nt):
                        nc.tensor.matmul(
                            out=ps[:], lhsT=ident_b[:], rhs=gb[:, j, :],
                            start=(j == 0), stop=(j == nt - 1),
                        )
                    nc.vector.tensor_scalar_mul(
                        o1_all[:, w, :], ps[:], dismy_s[:, w : w + 1]
                    )
                    o1b = sqpool.tile([P, D1], bf16, tag="o1b")
                    nc.vector.tensor_copy(out=o1b[:], in_=o1_all[:, w, :])
                    sq = sqpool.tile([P, D1], bf16, tag="sq")
                    nc.vector.tensor_mul(sq[:], o1_all[:, w, :], o1_all[:, w, :])
                    mcol = mask_b[:, 1:2] if w == WPC - 1 else mask_b[:, 0:1]
                    nc.tensor.matmul(
                        out=st_s[:], lhsT=mcol, rhs=o1b[:],
                        start=(w == 0), stop=(w == WPC - 1), skip_group_check=True,
                    )
                    nc.tensor.matmul(
                        out=st_q[:], lhsT=mcol, rhs=sq[:],
                        start=(w == 0), stop=(w == WPC - 1), skip_group_check=True,
                    )
                # stats -> DRAM -> AllReduce
                stats1 = o1_pool.tile([1, 2 * D1], f32)
                nc.vector.tensor_copy(out=stats1[:, 0:D1], in_=st_s[:])
                nc.vector.tensor_copy(out=stats1[:, D1:], in_=st_q[:])
            if phases < 3:
                outst = cpool.tile([P, WPC, D2], f32)
                nc.vector.tensor_copy(out=outst[:], in_=o1_all[:, :, 0:D2])
                nc.gpsimd.dma_start(
                    out=out_d[0:SLOTS, :].rearrange("(t p) d -> p t d", p=P),
                    in_=outst[:],
                )
                return nc
            nc.gpsimd.dma_start(out=ar1_in[:, :], in_=stats1[:])
            nc.gpsimd.collective_compute(
                "AllReduce", mybir.AluOpType.add,
                ins=[ar1_in[:, :]], outs=[ar1_out[:, :]], replica_groups=RG,
            )

            # ---------------- BN1 factors + h2 shard ----------------
            bnp = es.enter_context(tc.tile_pool(name="bn1", bufs=1))
            sg = bnp.tile([1, 2 * D1], f32)
            nc.sync.dma_start(out=sg[:], in_=ar1_out[:, :])
            mean = bnp.tile([1, D1], f32)
            nc.vector.tensor_scalar_mul(mean[:], sg[:, 0:D1], 1.0 / N)
            ex2 = bnp.tile([1, D1], f32)
            nc.vector.tensor_scalar_mul(ex2[:], sg[:, D1:], 1.0 / N)
            var = bnp.tile([1, D1], f32)
            nc.vector.tensor_mul(var[:], mean[:], mean[:])
            nc.vector.tensor_sub(var[:], ex2[:], var[:])
            epst = bnp.tile([1, 1], f32)
            nc.vector.memset(epst[:], EPS)
            sd = bnp.tile([1, D1], f32)
            nc.scalar.activation(sd[:], var[:], AF.Sqrt, bias=epst[:])
            rstd = bnp.tile([1, D1], f32)
            nc.vector.reciprocal(rstd[:], sd[:])
            a1 = bnp.tile([1, D1], f32)
            nc.vector.tensor_mul(a1[:], rstd[:], gb_s[:, 0:D1])
            c1 = bnp.tile([1, D1], f32)
            nc.vector.tensor_mul(c1[:], mean[:], a1[:])
            nc.vector.tensor_sub(c1[:], gb_s[:, D1 : 2 * D1], c1[:])
            # transpose (a1, c1) -> per-partition chunks [128, 2] per KC2 chunk
            acT = bnp.tile([P, KC2, 2], f32)
            with tc.tile_pool(name="trp", bufs=4, space="PSUM") as trpool:
                for c in range(KC2):
                    tpa = trpool.tile([P, 1], f32, tag="tra")
                    nc.tensor.transpose(
                        out=tpa[:], in_=a1[:, c * P : (c + 1) * P],
                        identity=ident_f[0:1, 0:1],
                    )
                    nc.vector.tensor_copy(out=acT[:, c, 0:1], in_=tpa[:])
                    tpc = trpool.tile([P, 1], f32, tag="trc")
                    nc.tensor.transpose(
                        out=tpc[:], in_=c1[:, c * P : (c + 1) * P],
                        identity=ident_f[0:1, 0:1],
                    )
                    nc.vector.tensor_copy(out=acT[:, c, 1:2], in_=tpc[:])

            # per window: transpose o1 chunk, BN+ReLU on ACT, W2 matmul, stage h2
            h2stage = bnp.tile([P, WPC, D2], bf16)
            with (
                tc.tile_pool(name="bnr", bufs=4) as bpool,
                tc.tile_pool(name="h2p", bufs=2, space="PSUM") as h2pool,
                tc.tile_pool(name="trq", bufs=4, space="PSUM") as trq,
            ):
                for w in range(WPC):
                    h2ps = h2pool.tile([P, D2], f32, tag="h2ps")
                    for c in range(KC2):
                        tp = trq.tile([P, P], f32, tag="tr")
                        nc.tensor.transpose(
                            out=tp[:], in_=o1_all[:, w, c * P : (c + 1) * P],
                            identity=ident_f[:],
                        )
                        bnr = bpool.tile([P, P], bf16, tag="bnr")
                        nc.scalar.activation(
                            bnr[:], tp[:], AF.Relu,
                            bias=acT[:, c, 1:2], scale=acT[:, c, 0:1],
                        )
                        nc.tensor.matmul(
                            out=h2ps[:], lhsT=bnr[:], rhs=w2_s[:, c, :],
                            start=(c == 0), stop=(c == KC2 - 1),
                        )
                    nc.vector.tensor_scalar_mul(
                        h2stage[:, w, :], h2ps[:], dismy_s[:, w : w + 1]
                    )
            zrow2 = bnp.tile([1, D2], bf16)
            nc.vector.memset(zrow2[:], 0)
            nc.gpsimd.dma_start(
                out=h2shard[0:SLOTS, :].rearrange("(t p) d -> p t d", p=P),
                in_=h2stage[:],
            )
            nc.gpsimd.dma_start(out=h2shard[SLOTS:BLK, :], in_=zrow2[:])
            nc.gpsimd.collective_compute(
                "AllGather", mybir.AluOpType.bypass,
                ins=[h2shard[:, :]], outs=[h2tab[:, :]], replica_groups=RG,
            )
            if phases < 4:
                outst = cpool.tile([P, WPC, D2], f32)
                nc.vector.tensor_copy(out=outst[:], in_=h2stage[:])
                nc.gpsimd.dma_start(
                    out=out_d[0:SLOTS, :].rearrange("(t p) d -> p t d", p=P),
                    in_=outst[:],
                )
                return nc

            # ---------------- conv2 ----------------
            o2_pool = es.enter_context(tc.tile_pool(name="o2", bufs=1))
            o2_all = o2_pool.tile([P, WPC, D2], f32)
            lo2 = h2tab[0:HB, :]
            hi2 = h2tab[HB:NROWS, :]
            with (
                tc.tile_pool(name="g2", bufs=2) as gpool2,
                tc.tile_pool(name="sq2", bufs=2) as sqpool2,
                tc.tile_pool(name="c2p", bufs=2, space="PSUM") as wpool2,
                tc.tile_pool(name="st2p", bufs=1, space="PSUM") as stpool2,
            ):
                st2_s = stpool2.tile([1, D2], f32, tag="st2_s")
                st2_q = stpool2.tile([1, D2], f32, tag="st2_q")
                for w in range(WPC):
                    nl, nh = int(NL[w]), int(NH[w])
                    nt = nl + nh
                    gb = gpool2.tile([P, NTmax, D2], bf16, tag="g2")
                    if nl:
                        nc.gpsimd.dma_gather(
                            gb[:, 0:nl, :], lo2, idx_s[:, offL[w] // 16 : offL[w] // 16 + nl * 8],
                            nl * P, nl * P, D2, elem_step=D2, single_packet=False,
                        )
                    if nh:
                        nc.gpsimd.dma_gather(
                            gb[:, nl:nt, :], hi2, idx_s[:, offH[w] // 16 : offH[w] // 16 + nh * 8],
                            nh * P, nh * P, D2, elem_step=D2, single_packet=False,
                        )
                    ps = wpool2.tile([P, D2], f32, tag="win2")
                    for j in range(nt):
                        nc.tensor.matmul(
                            out=ps[:], lhsT=ident_b[:], rhs=gb[:, j, :],
                            start=(j == 0), stop=(j == nt - 1),
                        )
                    nc.vector.tensor_scalar_mul(
                        o2_all[:, w, :], ps[:], dismy_s[:, w : w + 1]
                    )
                    o2b = sqpool2.tile([P, D2], bf16, tag="o2b")
                    nc.vector.tensor_copy(out=o2b[:], in_=o2_all[:, w, :])
                    sq = sqpool2.tile([P, D2], bf16, tag="sq2")
                    nc.vector.tensor_mul(sq[:], o2_all[:, w, :], o2_all[:, w, :])
                    mcol = mask_b[:, 1:2] if w == WPC - 1 else mask_b[:, 0:1]
                    nc.tensor.matmul(
                        out=st2_s[:], lhsT=mcol, rhs=o2b[:],
                        start=(w == 0), stop=(w == WPC - 1), skip_group_check=True,
                    )
                    nc.tensor.matmul(
                        out=st2_q[:], lhsT=mcol, rhs=sq[:],
                        start=(w == 0), stop=(w == WPC - 1), skip_group_check=True,
                    )
                stats2 = o2_pool.tile([1, 2 * D2], f32)
                nc.vector.tensor_copy(out=stats2[:, 0:D2], in_=st2_s[:])
                nc.vector.tensor_copy(out=stats2[:, D2:], in_=st2_q[:])
            if phases < 5:
                outst = cpool.tile([P, WPC, D2], f32)
                nc.vector.tensor_copy(out=outst[:], in_=o2_all[:, :, :])
                nc.gpsimd.dma_start(
                    out=out_d[0:SLOTS, :].rearrange("(t p) d -> p t d", p=P),
                    in_=outst[:],
                )
                return nc
            nc.gpsimd.dma_start(out=ar2_in[:, :], in_=stats2[:])
            nc.gpsimd.collective_compute(
                "AllReduce", mybir.AluOpType.add,
                ins=[ar2_in[:, :]], outs=[ar2_out[:, :]], replica_groups=RG,
            )

            # ---------------- BN2 + output ----------------
            sg2 = o2_pool.tile([1, 2 * D2], f32)
            nc.sync.dma_start(out=sg2[:], in_=ar2_out[:, :])
            mean2 = o2_pool.tile([1, D2], f32)
            nc.vector.tensor_scalar_mul(mean2[:], sg2[:, 0:D2], 1.0 / N)
            ex22 = o2_pool.tile([1, D2], f32)
            nc.vector.tensor_scalar_mul(ex22[:], sg2[:, D2:], 1.0 / N)
            var2 = o2_pool.tile([1, D2], f32)
            nc.vector.tensor_mul(var2[:], mean2[:], mean2[:])
            nc.vector.tensor_sub(var2[:], ex22[:], var2[:])
            epst2 = o2_pool.tile([1, 1], f32)
            nc.vector.memset(epst2[:], EPS)
            sd2 = o2_pool.tile([1, D2], f32)
            nc.scalar.activation(sd2[:], var2[:], AF.Sqrt, bias=epst2[:])
            rstd2 = o2_pool.tile([1, D2], f32)
            nc.vector.reciprocal(rstd2[:], sd2[:])
            a2 = o2_pool.tile([1, D2], f32)
            nc.vector.tensor_mul(a2[:], rstd2[:], gb_s[:, 2 * D1 : 2 * D1 + D2])
            c2 = o2_pool.tile([1, D2], f32)
            nc.vector.tensor_mul(c2[:], mean2[:], a2[:])
            nc.vector.tensor_sub(c2[:], gb_s[:, 2 * D1 + D2 :], c2[:])

            # broadcast a2/c2 across partitions via ones-column matmul
            onesrow = o2_pool.tile([1, P], f32)
            nc.vector.memset(onesrow[:], 1.0)
            a2b = o2_pool.tile([P, D2], f32)
            c2b = o2_pool.tile([P, D2], f32)
            with tc.tile_pool(name="bn2p", bufs=2, space="PSUM") as bn2p:
                bps = bn2p.tile([P, D2], f32, tag="b2a")
                nc.tensor.matmul(out=bps[:], lhsT=onesrow[:], rhs=a2[:], start=True, stop=True)
                nc.vector.tensor_copy(out=a2b[:], in_=bps[:])
                cps = bn2p.tile([P, D2], f32, tag="b2c")
                nc.tensor.matmul(out=cps[:], lhsT=onesrow[:], rhs=c2[:], start=True, stop=True)
                nc.vector.tensor_copy(out=c2b[:], in_=cps[:])

            outst = o2_pool.tile([P, WPC, D2], f32)
            for w in range(WPC):
                nc.vector.tensor_mul(outst[:, w, :], o2_all[:, w, :], a2b[:])
                nc.vector.tensor_add(outst[:, w, :], outst[:, w, :], c2b[:])
            nc.gpsimd.dma_start(
                out=out_d[0:SLOTS, :].rearrange("(t p) d -> p t d", p=P),
                in_=outst[:],
            )

    return nc


# ---------------------------------------------------------------- entry point

def _run(x, edge_index, W1, gamma1, beta1, W2, gamma2, beta2, cfg, trace=False):
    from concourse.bass_utils import run_bass_kernel_spmd

    N = cfg["N"]
    pp = _preprocess(edge_index, N)
    xb, w1b, w2b = _pack_inputs(np.asarray(x, np.float32), np.asarray(W1, np.float32),
                                np.asarray(W2, np.float32), pp, cfg)
    nc = _build_kernel(cfg, pp, phases=int(__import__("os").environ.get("K_PHASES", "5")))
    nc.compile()

    shared = {
        "xb": np.ascontiguousarray(xb),
        "w1b": np.ascontiguousarray(w1b),
        "w2b": np.ascontiguousarray(w2b),
        "disall": np.ascontiguousarray(pp["disall"]),
        "statmask": np.ascontiguousarray(pp["statmask"]),
        "gamma1": np.asarray(gamma1, np.float32).reshape(1, -1),
        "beta1": np.asarray(beta1, np.float32).reshape(1, -1),
        "gamma2": np.asarray(gamma2, np.float32).reshape(1, -1),
        "beta2": np.asarray(beta2, np.float32).reshape(1, -1),
    }
    in_maps = []
    for c in range(N_CORES):
        m = dict(shared)
        m["idx"] = np.ascontiguousarray(pp["idx_wrapped"][c])
        m["dismy"] = np.ascontiguousarray(pp["dismy"][c])
        in_maps.append(m)

    res = run_bass_kernel_spmd(nc, in_maps, core_ids=list(range(N_CORES)), trace=trace)

    D2 = cfg["D2"]
    out = np.empty((N, D2), np.float32)
    pos, core_of = pp["pos"], pp["core_of"]
    for c in range(N_CORES):
        nodes = np.flatnonzero(core_of == c)
        out[nodes] = res.results[c]["out"][pos[nodes]]
    _run.last_result = res
    return out


def kernel(x, edge_index, W1, b1, gamma1, beta1, W2, b2, gamma2, beta2):
    # b1/b2 cancel exactly through BatchNorm's mean subtraction; unused.
    return _run(x, edge_index, W1, gamma1, beta1, W2, gamma2, beta2, _FULL_CFG)


# revision 14
# speedup vs baseline: 455.8801x; 455.8801x over previous
"""GCN encoder (2x GCNConv + BatchNorm + ReLU) on 8 Trainium2 NeuronCores.

Strategy (graph/data parallel, per sharding hint):
- Nodes are permuted (degree-sorted, round-robin dealt) and sharded across the
  8 cores; each core owns 49 "windows" of 128 destination nodes.
- norm factorizes: norm(s,d) = dis[s]*dis[d].  Source scaling dis[s] is folded
  into the feature tables (h~ = dis * h); destination scaling dis[d] is applied
  on PSUM eviction.  Messages then aggregate with a *constant identity* matmul:
  for each window, gathered source rows land in "slots" (slot = local dst id),
  and chunk matmuls with a preloaded identity accumulate them in PSUM.
- Gathers use the int16 dma_gather embedding path.  int16 limits addressing to
  32768 rows, so the feature table is split in two halves (cores 0-3 / 4-7) and
  each window runs one gather per half; pad slots point at an all-zero row.
- h1 = x @ W1 is computed redundantly on every core (full table in local HBM).
  h2's input depends on BN1 (global stats -> AllReduce); each core computes its
  own shard of h2 = relu(bn(conv1)) @ W2 and an AllGather replicates the table.
- BatchNorm uses E[x^2]-mean^2 with sums computed by ones-vector matmuls
  (partition reduction) accumulated in PSUM across windows, then AllReduce.
- b1/b2 are ignored: a per-feature constant added before BatchNorm cancels
  exactly in (x - mean).
"""

import sys

sys.path.insert(0, "/opt/trn_rl_repo")

import numpy as np

N_CORES = 8
P = 128
EPS = 1e-5

_FULL_CFG = dict(N=50000, IN=512, D1=256, D2=128)


# ---------------------------------------------------------------- host preprocessing

def _preprocess(edge_index, N):
    """Graph preprocessing: node permutation, slot/chunk assignment, gather
    indices.  Pure integer work on the host."""
    src = np.asarray(edge_index[0], dtype=np.int64)
    dst = np.asarray(edge_index[1], dtype=np.int64)
    # append self loops
    loop = np.arange(N, dtype=np.int64)
    S = np.concatenate([src, loop])
    D = np.concatenate([dst, loop])

    deg = np.bincount(D, minlength=N)  # >= 1 (self loop)
    dis = (1.0 / np.sqrt(deg.astype(np.float64))).astype(np.float32)

    real_pc = N // N_CORES
    WPC = (real_pc + P - 1) // P          # windows per core
    SLOTS = WPC * P                        # slot positions per core
    BLK = SLOTS + 1                        # +1 trailing zero row per core block

    # deal nodes to cores round-robin in degree-desc order
    order = np.argsort(-deg, kind="stable")
    core_of = np.empty(N, dtype=np.int64)
    core_of[order] = np.arange(N) % N_CORES

    half_node = core_of >= (N_CORES // 2)  # False = lo table half
    halfE = half_node[S]

    deg_lo = np.bincount(D[~halfE], minlength=N)
    deg_hi = deg - deg_lo

    # position of each node within its core: sort by (deg_lo desc, deg_hi desc),
    # then re-sort blocks of 768 by deg_hi — keeps per-window max(deg_lo) and
    # max(deg_hi) both tight (gather padding ~18% instead of ~31%)
    pos = np.empty(N, dtype=np.int64)
    node_by_cp = np.full((N_CORES, SLOTS), -1, dtype=np.int64)
    RESORT_BLK = 768
    for c in range(N_CORES):
        nodes_c = np.flatnonzero(core_of == c)
        o = np.lexsort((-deg_hi[nodes_c], -deg_lo[nodes_c]))
        for s in range(0, len(o), RESORT_BLK):
            blk = o[s : s + RESORT_BLK]
            o[s : s + RESORT_BLK] = blk[np.argsort(-deg_hi[nodes_c][blk], kind="stable")]
        snodes = nodes_c[o]
        pos[snodes] = np.arange(len(snodes))
        node_by_cp[c, : len(snodes)] = snodes

    # per-core per-window chunk counts -> global max (SPMD static shapes)
    dlo_cp = np.zeros((N_CORES, SLOTS), dtype=np.int64)
    dhi_cp = np.zeros((N_CORES, SLOTS), dtype=np.int64)
    m = node_by_cp >= 0
    dlo_cp[m] = deg_lo[node_by_cp[m]]
    dhi_cp[m] = deg_hi[node_by_cp[m]]
    NLc = dlo_cp.reshape(N_CORES, WPC, P).max(axis=2)
    NHc = dhi_cp.reshape(N_CORES, WPC, P).max(axis=2)
    NL = NLc.max(axis=0)
    NH = NHc.max(axis=0)

    # idx segment offsets: per window [lo seg][hi seg], chunk-major inside
    seg = (NL + NH) * P
    base = np.concatenate([[0], np.cumsum(seg)])
    offL = base[:-1]
    offH = base[:-1] + NL * P
    TOT = int(base[-1])

    # chunk index of each edge among its (dst, half) group
    key = D * 2 + halfE
    ksort = np.argsort(key, kind="stable")
    skey = key[ksort]
    starts = np.concatenate([[0], np.flatnonzero(np.diff(skey)) + 1])
    group_len = np.diff(np.concatenate([starts, [len(skey)]]))
    chunk_sorted = np.arange(len(skey)) - np.repeat(starts, group_len)
    chunk = np.empty(len(S), dtype=np.int64)
    chunk[ksort] = chunk_sorted

    cD = core_of[D]
    wD = pos[D] // P
    slotD = pos[D] % P
    absrow = core_of[S] * BLK + pos[S]
    rel = np.where(~halfE, absrow, absrow - (N_CORES // 2) * BLK)
    assert rel.max() < 32768
    epos = np.where(~halfE, offL[wD], offH[wD]) + chunk * P + slotD

    PADIDX = SLOTS  # each block's trailing zero row (rel within half view)
    flat = np.full(N_CORES * TOT, PADIDX, dtype=np.int16)
    flat[cD * TOT + epos] = rel.astype(np.int16)
    flat = flat.reshape(N_CORES, TOT)
    # wrap: idx i -> [i%16, i//16], replicated across the 8 groups of 16 rows
    wrapped16 = flat.reshape(N_CORES, TOT // 16, 16).transpose(0, 2, 1)
    idx_wrapped = np.tile(wrapped16, (1, P // 16, 1))  # [cores, 128, TOT/16]

    # per-core dis (by slot), 1.0 for dummies
    dis_cp = np.ones((N_CORES, SLOTS), dtype=np.float32)
    dis_cp[m] = dis[node_by_cp[m]]
    dismy = dis_cp.reshape(N_CORES, WPC, P).transpose(0, 2, 1)  # [c, 128, WPC]

    # global dis in table-data-row order (for phase 1), [128, NTILES]
    NTILES = N_CORES * WPC
    disall = dis_cp.reshape(NTILES, P).T.copy()  # [128, NTILES]

    # stats mask: last window has (SLOTS - real_pc) dummy rows at the end
    n_dummy = SLOTS - real_pc
    statmask = np.ones((P, 2), dtype=np.float32)
    if n_dummy:
        statmask[P - n_dummy :, 1] = 0.0

    waste = float(seg.sum()) / max(1, len(S) / N_CORES) - 1.0
    return dict(
        WPC=WPC, SLOTS=SLOTS, BLK=BLK, NTILES=NTILES,
        NL=NL.astype(int), NH=NH.astype(int), TOT=TOT,
        offL=offL, offH=offH,
        idx_wrapped=idx_wrapped, dismy=dismy, disall=disall,
        statmask=statmask, node_by_cp=node_by_cp, pos=pos, core_of=core_of,
        dis=dis, real_pc=real_pc, waste=waste,
    )


def _pack_inputs(x, W1, W2, pp, cfg):
    """Build the per-core / shared device input arrays."""
    import ml_dtypes

    bf16 = ml_dtypes.bfloat16
    N, IN, D1, D2 = cfg["N"], cfg["IN"], cfg["D1"], cfg["D2"]
    WPC, SLOTS, NTILES = pp["WPC"], pp["SLOTS"], pp["NTILES"]
    KC = IN // P

    # x permuted to table order (pre-scaled by dis so h~ = (dis*x) @ W1),
    # zeros for dummies -> [NTILES, 128(p=k), KC, 128(j=row)]
    xperm = np.zeros((N_CORES * SLOTS, IN), dtype=np.float32)
    m = pp["node_by_cp"] >= 0
    xperm[m.reshape(-1)] = x[pp["node_by_cp"][m]] * pp["dis"][pp["node_by_cp"][m]][:, None]
    xb = (
        xperm.reshape(NTILES, P, KC, P)   # [b, j, kc, p]
        .transpose(0, 3, 2, 1)            # [b, p, kc, j]
        .astype(bf16)
    )
    w1b = W1.reshape(KC, P, D1).transpose(1, 0, 2).astype(bf16)   # [p, kc, D1]
    w2b = W2.reshape(D1 // P, P, D2).transpose(1, 0, 2).astype(bf16)  # [p, kc, D2]
    return xb, w1b, w2b


# ---------------------------------------------------------------- device kernel

def _build_kernel(cfg, pp, phases=5):
    import concourse.bacc as bacc
    import concourse.mybir as mybir
    import concourse.tile as tile
    from concourse.masks import make_identity
    from contextlib import ExitStack

    N, IN, D1, D2 = cfg["N"], cfg["IN"], cfg["D1"], cfg["D2"]
    WPC, SLOTS, BLK, NTILES = pp["WPC"], pp["SLOTS"], pp["BLK"], pp["NTILES"]
    NL, NH, TOT = pp["NL"], pp["NH"], pp["TOT"]
    offL, offH = pp["offL"], pp["offH"]
    KC = IN // P
    KC2 = D1 // P
    HB = (N_CORES // 2) * BLK        # hi half base row
    NROWS = N_CORES * BLK            # table rows
    NTmax = int((NL + NH).max())
    RG = [list(range(N_CORES))]
    f32, bf16, i16 = mybir.dt.float32, mybir.dt.bfloat16, mybir.dt.int16
    f32r = mybir.dt.float32r
    AF = mybir.ActivationFunctionType

    nc = bacc.Bacc(num_devices=N_CORES)

    # ---- I/O
    xb_d = nc.dram_tensor("xb", [NTILES, P, KC, P], bf16, kind="ExternalInput")
    w1_d = nc.dram_tensor("w1b", [P, KC, D1], bf16, kind="ExternalInput")
    w2_d = nc.dram_tensor("w2b", [P, KC2, D2], bf16, kind="ExternalInput")
    idx_d = nc.dram_tensor("idx", [P, TOT // 16], i16, kind="ExternalInput")
    dismy_d = nc.dram_tensor("dismy", [P, WPC], f32, kind="ExternalInput")
    mask_d = nc.dram_tensor("statmask", [P, 2], f32, kind="ExternalInput")
    g1_d = nc.dram_tensor("gamma1", [1, D1], f32, kind="ExternalInput")
    b1_d = nc.dram_tensor("beta1", [1, D1], f32, kind="ExternalInput")
    g2_d = nc.dram_tensor("gamma2", [1, D2], f32, kind="ExternalInput")
    b2_d = nc.dram_tensor("beta2", [1, D2], f32, kind="ExternalInput")
    out_d = nc.dram_tensor("out", [SLOTS, D2], f32, kind="ExternalOutput")

    # ---- internal DRAM
    h1tab = nc.dram_tensor("h1tab", [NROWS, D1], bf16, kind="Internal")
    h2shard = nc.dram_tensor("h2shard", [BLK, D2], bf16, kind="Internal")
    h2tab = nc.dram_tensor("h2tab", [NROWS, D2], bf16, kind="Internal", addr_space="Shared")
    ar1_in = nc.dram_tensor("ar1_in", [1, 2 * D1], f32, kind="Internal")
    ar1_out = nc.dram_tensor("ar1_out", [1, 2 * D1], f32, kind="Internal", addr_space="Shared")
    ar2_in = nc.dram_tensor("ar2_in", [1, 2 * D2], f32, kind="Internal")
    ar2_out = nc.dram_tensor("ar2_out", [1, 2 * D2], f32, kind="Internal", addr_space="Shared")

    import concourse.bass as bass

    def pad_rows_ap(tensor, D):
        # rows {c*BLK + SLOTS : c in 0..7} of a [NROWS, D] table
        return bass.AP(tensor, SLOTS * D, [[BLK * D, N_CORES], [1, D]])

    with tile.TileContext(nc) as tc:
        es = ExitStack()
        with es:
            cpool = es.enter_context(tc.tile_pool(name="const", bufs=1))
            ident_b = cpool.tile([P, P], bf16)
            make_identity(nc, ident_b[:])
            ident_f = cpool.tile([P, P], f32)
            make_identity(nc, ident_f[:])
            w1_s = cpool.tile([P, KC, D1], bf16)
            nc.sync.dma_start(out=w1_s[:], in_=w1_d[:, :, :])
            w2_s = cpool.tile([P, KC2, D2], bf16)
            nc.sync.dma_start(out=w2_s[:], in_=w2_d[:, :, :])
            dismy_s = cpool.tile([P, WPC], f32)
            nc.sync.dma_start(out=dismy_s[:], in_=dismy_d[:, :])
            mask_s = cpool.tile([P, 2], f32)
            nc.sync.dma_start(out=mask_s[:], in_=mask_d[:, :])
            mask_b = cpool.tile([P, 2], bf16)
            nc.vector.tensor_copy(out=mask_b[:], in_=mask_s[:])
            idx_s = cpool.tile([P, TOT // 16], i16)
            nc.sync.dma_start(out=idx_s[:], in_=idx_d[:, :])
            gb_s = cpool.tile([1, 2 * D1 + 2 * D2], f32)  # gamma1|beta1|gamma2|beta2
            nc.sync.dma_start(out=gb_s[:, 0:D1], in_=g1_d[:, :])
            nc.sync.dma_start(out=gb_s[:, D1 : 2 * D1], in_=b1_d[:, :])
            nc.sync.dma_start(out=gb_s[:, 2 * D1 : 2 * D1 + D2], in_=g2_d[:, :])
            nc.sync.dma_start(out=gb_s[:, 2 * D1 + D2 :], in_=b2_d[:, :])

            # zero pad rows of h1tab (one strided DMA)
            zrow = cpool.tile([N_CORES, D1], bf16)
            nc.vector.memset(zrow[:], 0)
            nc.gpsimd.dma_start(out=pad_rows_ap(h1tab, D1), in_=zrow[:])

            # ---------------- phase 1: h1tab = dis * (x @ W1), all rows ----------------
            with (
                tc.tile_pool(name="p1x", bufs=4) as xpool,
                tc.tile_pool(name="p1s", bufs=2) as spool,
                tc.tile_pool(name="p1p", bufs=4, space="PSUM") as ppool1,
            ):
                XB = 7 if WPC % 7 == 0 else 1   # x tiles per DMA
                for blk in range(N_CORES):
                    stage = spool.tile([P, WPC, D1], bf16, tag="stage")
                    for tb in range(WPC // XB):
                        b0 = blk * WPC + tb * XB
                        xt = xpool.tile([P, XB, KC, P], bf16, tag="xt")
                        nc.sync.dma_start(
                            out=xt[:], in_=xb_d[b0 : b0 + XB].rearrange("b p k j -> p b k j")
                        )
                        for t2 in range(XB):
                            t = tb * XB + t2
                            ps = ppool1.tile([P, D1], f32, tag="ps1")
                            for kc in range(KC):
                                nc.tensor.matmul(
                                    out=ps[:], lhsT=xt[:, t2, kc, :], rhs=w1_s[:, kc, :],
                                    start=(kc == 0), stop=(kc == KC - 1),
                                )
                            nc.scalar.activation(
                                stage[:, t, :], ps[:], AF.Copy
                            )
                    nc.gpsimd.dma_start(
                        out=h1tab[blk * BLK : blk * BLK + SLOTS, :].rearrange(
                            "(t p) d -> p t d", p=P
                        ),
                        in_=stage[:],
                    )

            # ---------------- conv1: window aggregation ----------------
            if phases < 2:
                outst = cpool.tile([P, WPC, D2], f32)
                nc.vector.memset(outst[:], 0)
                nc.gpsimd.dma_start(
                    out=out_d[0:SLOTS, :].rearrange("(t p) d -> p t d", p=P),
                    in_=outst[:],
                )
                return nc
            o1_pool = es.enter_context(tc.tile_pool(name="o1", bufs=1))
            o1_all = o1_pool.tile([P, WPC, D1], f32)
            lo_view = h1tab[0:HB, :]
            hi_view = h1tab[HB:NROWS, :]
            with (
                tc.tile_pool(name="g1", bufs=2) as gpool,
                tc.tile_pool(name="sq1", bufs=2) as sqpool,
                tc.tile_pool(name="c1p", bufs=3, space="PSUM") as wpool,
                tc.tile_pool(name="st1p", bufs=1, space="PSUM") as stpool,
            ):
                st_s = stpool.tile([1, D1], f32, tag="st_s")
                st_q = stpool.tile([1, D1], f32, tag="st_q")
                for w in range(WPC):
                    nl, nh = int(NL[w]), int(NH[w])
                    nt = nl + nh
                    gb = gpool.tile([P, NTmax, D1], bf16, tag="g1")
                    if nl:
                        nc.gpsimd.dma_gather(
                            gb[:, 0:nl, :], lo_view, idx_s[:, offL[w] // 16 : offL[w] // 16 + nl * 8],
                            nl * P, nl * P, D1, elem_step=D1, single_packet=False,
                        )
                    if nh:
                        nc.gpsimd.dma_gather(
                            gb[:, nl:nt, :], hi_view, idx_s[:, offH[w] // 16 : offH[w] // 16 + nh * 8],
                            nh * P, nh * P, D1, elem_step=D1, single_packet=False,
                        )
                    ps = wpool.tile([P, D1], f32, tag="win1")
                    for j in range(nt):
                        nc.tensor.matmul(
                            out=ps[:], lhsT=ident_b[:], rhs=gb[:, j, :],
                            start=(j == 0), stop=(j == nt - 1),
                        )
                    nc.vector.tensor_scalar_mul(
                        o1_all[:, w, :], ps[:], dismy_s[:, w : w + 1]
                    )
                    o1b = sqpool.tile([P, D1], bf16, tag="o1b")
                    nc.vector.tensor_copy(out=o1b[:], in_=o1_all[:, w, :])
                    sq = sqpool.tile([P, D1], bf16, tag="sq")
                    nc.vector.tensor_mul(sq[:], o1_all[:, w, :], o1_all[:, w, :])
                    mcol = mask_b[:, 1:2] if w == WPC - 1 else mask_b[:, 0:1]
                    nc.tensor.matmul(
                        out=st_s[:], lhsT=mcol, rhs=o1b[:],
                        start=(w == 0), stop=(w == WPC - 1), skip_group_check=True,
                    )
                    nc.tensor.matmul(
                        out=st_q[:], lhsT=mcol, rhs=sq[:],
                        start=(w == 0), stop=(w == WPC - 1), skip_group_check=True,
                    )
                # stats -> DRAM -> AllReduce
                stats1 = o1_pool.tile([1, 2 * D1], f32)
                nc.vector.tensor_copy(out=stats1[:, 0:D1], in_=st_s[:])
                nc.vector.tensor_copy(out=stats1[:, D1:], in_=st_q[:])
            if phases < 3:
                outst = cpool.tile([P, WPC, D2], f32)
                nc.vector.tensor_copy(out=outst[:], in_=o1_all[:, :, 0:D2])
                nc.gpsimd.dma_start(
                    out=out_d[0:SLOTS, :].rearrange("(t p) d -> p t d", p=P),
                    in_=outst[:],
                )
                return nc
            nc.gpsimd.dma_start(out=ar1_in[:, :], in_=stats1[:])
            nc.gpsimd.collective_compute(
                "AllReduce", mybir.AluOpType.add,
                ins=[ar1_in[:, :]], outs=[ar1_out[:, :]], replica_groups=RG,
            )

            # ---------------- BN1 factors + h2 shard ----------------
            bnp = es.enter_context(tc.tile_pool(name="bn1", bufs=1))
            sg = bnp.tile([1, 2 * D1], f32)
            nc.sync.dma_start(out=sg[:], in_=ar1_out[:, :])
            mean = bnp.tile([1, D1], f32)
            nc.vector.tensor_scalar_mul(mean[:], sg[:, 0:D1], 1.0 / N)
            ex2 = bnp.tile([1, D1], f32)
            nc.vector.tensor_scalar_mul(ex2[:], sg[:, D1:], 1.0 / N)
            var = bnp.tile([1, D1], f32)
            nc.vector.tensor_mul(var[:], mean[:], mean[:])
            nc.vector.tensor_sub(var[:], ex2[:], var[:])
            epst = bnp.tile([1, 1], f32)
            nc.vector.memset(epst[:], EPS)
            sd = bnp.tile([1, D1], f32)
            nc.scalar.activation(sd[:], var[:], AF.Sqrt, bias=epst[:])
            rstd = bnp.tile([1, D1], f32)
            nc.vector.reciprocal(rstd[:], sd[:])
            a1 = bnp.tile([1, D1], f32)
            nc.vector.tensor_mul(a1[:], rstd[:], gb_s[:, 0:D1])
            c1 = bnp.tile([1, D1], f32)
            nc.vector.tensor_mul(c1[:], mean[:], a1[:])
            nc.vector.tensor_sub(c1[:], gb_s[:, D1 : 2 * D1], c1[:])
            # transpose (a1, c1) -> per-partition chunks [128, 2] per KC2 chunk
            acT = bnp.tile([P, KC2, 2], f32)
            with tc.tile_pool(name="trp", bufs=4, space="PSUM") as trpool:
                for c in range(KC2):
                    tpa = trpool.tile([P, 1], f32, tag="tra")
                    nc.tensor.transpose(
                        out=tpa[:], in_=a1[:, c * P : (c + 1) * P],
                        identity=ident_f[0:1, 0:1],
                    )
                    nc.vector.tensor_copy(out=acT[:, c, 0:1], in_=tpa[:])
                    tpc = trpool.tile([P, 1], f32, tag="trc")
                    nc.tensor.transpose(
                        out=tpc[:], in_=c1[:, c * P : (c + 1) * P],
                        identity=ident_f[0:1, 0:1],
                    )
                    nc.vector.tensor_copy(out=acT[:, c, 1:2], in_=tpc[:])

            # per window: transpose o1 chunk, BN+ReLU on ACT, W2 matmul, stage h2
            h2stage = bnp.tile([P, WPC, D2], bf16)
            with (
                tc.tile_pool(name="bnr", bufs=4) as bpool,
                tc.tile_pool(name="h2p", bufs=2, space="PSUM") as h2pool,
                tc.tile_pool(name="trq", bufs=4, space="PSUM") as trq,
            ):
                for w in range(WPC):
                    h2ps = h2pool.tile([P, D2], f32, tag="h2ps")
                    for c in range(KC2):
                        tp = trq.tile([P, P], f32, tag="tr")
                        nc.tensor.transpose(
                            out=tp[:], in_=o1_all[:, w, c * P : (c + 1) * P],
                            identity=ident_f[:],
                        )
                        bnr = bpool.tile([P, P], bf16, tag="bnr")
                        nc.scalar.activation(
                            bnr[:], tp[:], AF.Relu,
                            bias=acT[:, c, 1:2], scale=acT[:, c, 0:1],
                        )
                        nc.tensor.matmul(
                            out=h2ps[:], lhsT=bnr[:], rhs=w2_s[:, c, :],
                            start=(c == 0), stop=(c == KC2 - 1),
                        )
                    nc.vector.tensor_scalar_mul(
                        h2stage[:, w, :], h2ps[:], dismy_s[:, w : w + 1]
                    )
            zrow2 = bnp.tile([1, D2], bf16)
            nc.vector.memset(zrow2[:], 0)
            nc.gpsimd.dma_start(
                out=h2shard[0:SLOTS, :].rearrange("(t p) d -> p t d", p=P),
                in_=h2stage[:],
            )
            nc.gpsimd.dma_start(out=h2shard[SLOTS:BLK, :], in_=zrow2[:])
            nc.gpsimd.collective_compute(
                "AllGather", mybir.AluOpType.bypass,
                ins=[h2shard[:, :]], outs=[h2tab[:, :]], replica_groups=RG,
            )
            if phases < 4:
                outst = cpool.tile([P, WPC, D2], f32)
                nc.vector.tensor_copy(out=outst[:], in_=h2stage[:])
                nc.gpsimd.dma_start(
                    out=out_d[0:SLOTS, :].rearrange("(t p) d -> p t d", p=P),
                    in_=outst[:],
                )
                return nc

            # ---------------- conv2 ----------------
            o2_pool = es.enter_context(tc.tile_pool(name="o2", bufs=1))
            o2_all = o2_pool.tile([P, WPC, D2], f32)
            lo2 = h2tab[0:HB, :]
            hi2 = h2tab[HB:NROWS, :]
            with (
                tc.tile_pool(name="g2", bufs=2) as gpool2,
                tc.tile_pool(name="sq2", bufs=2) as sqpool2,
                tc.tile_pool(name="c2p", bufs=3, space="PSUM") as wpool2,
                tc.tile_pool(name="st2p", bufs=1, space="PSUM") as stpool2,
            ):
                st2_s = stpool2.tile([1, D2], f32, tag="st2_s")
                st2_q = stpool2.tile([1, D2], f32, tag="st2_q")
                for w in range(WPC):
                    nl, nh = int(NL[w]), int(NH[w])
                    nt = nl + nh
                    gb = gpool2.tile([P, NTmax, D2], bf16, tag="g2")
                    if nl:
                        nc.gpsimd.dma_gather(
                            gb[:, 0:nl, :], lo2, idx_s[:, offL[w] // 16 : offL[w] // 16 + nl * 8],
                            nl * P, nl * P, D2, elem_step=D2, single_packet=False,
                        )
                    if nh:
                        nc.gpsimd.dma_gather(
                            gb[:, nl:nt, :], hi2, idx_s[:, offH[w] // 16 : offH[w] // 16 + nh * 8],
                            nh * P, nh * P, D2, elem_step=D2, single_packet=False,
                        )
                    ps = wpool2.tile([P, D2], f32, tag="win2")
                    for j in range(nt):
                        nc.tensor.matmul(
                            out=ps[:], lhsT=ident_b[:], rhs=gb[:, j, :],
                            start=(j == 0), stop=(j == nt - 1),
                        )
                    nc.vector.tensor_scalar_mul(
                        o2_all[:, w, :], ps[:], dismy_s[:, w : w + 1]
                    )
                    o2b = sqpool2.tile([P, D2], bf16, tag="o2b")
                    nc.vector.tensor_copy(out=o2b[:], in_=o2_all[:, w, :])
                    sq = sqpool2.tile([P, D2], bf16, tag="sq2")
                    nc.vector.tensor_mul(sq[:], o2_all[:, w, :], o2_all[:, w, :])
                    mcol = mask_b[:, 1:2] if w == WPC - 1 else mask_b[:, 0:1]
                    nc.tensor.matmul(
                        out=st2_s[:], lhsT=mcol, rhs=o2b[:],
                        start=(w == 0), stop=(w == WPC - 1), skip_group_check=True,
                    )
                    nc.tensor.matmul(
                        out=st2_q[:], lhsT=mcol, rhs=sq[:],
                        start=(w == 0), stop=(w == WPC - 1), skip_group_check=True,
                    )
                stats2 = o2_pool.tile([1, 2 * D2], f32)
                nc.vector.tensor_copy(out=stats2[:, 0:D2], in_=st2_s[:])
                nc.vector.tensor_copy(out=stats2[:, D2:], in_=st2_q[:])
            if phases < 5:
                outst = cpool.tile([P, WPC, D2], f32)
                nc.vector.tensor_copy(out=outst[:], in_=o2_all[:, :, :])
                nc.gpsimd.dma_start(
                    out=out_d[0:SLOTS, :].rearrange("(t p) d -> p t d", p=P),
                    in_=outst[:],
                )
                return nc
            nc.gpsimd.dma_start(out=ar2_in[:, :], in_=stats2[:])
            nc.gpsimd.collective_compute(
                "AllReduce", mybir.AluOpType.add,
                ins=[ar2_in[:, :]], outs=[ar2_out[:, :]], replica_groups=RG,
            )

            # ---------------- BN2 + output ----------------
            sg2 = o2_pool.tile([1, 2 * D2], f32)
            nc.sync.dma_start(out=sg2[:], in_=ar2_out[:, :])
            mean2 = o2_pool.tile([1, D2], f32)
            nc.vector.tensor_scalar_mul(mean2[:], sg2[:, 0:D2], 1.0 / N)
            ex22 = o2_pool.tile([1, D2], f32)
            nc.vector.tensor_scalar_mul(ex22[:], sg2[:, D2:], 1.0 / N)
            var2 = o2_pool.tile([1, D2], f32)
            nc.vector.tensor_mul(var2[:], mean2[:], mean2[:])
            nc.vector.tensor_sub(var2[:], ex22[:], var2[:])
            epst2 = o2_pool.tile([1, 1], f32)
            nc.vector.memset(epst2[:], EPS)
            sd2 = o2_pool.tile([1, D2], f32)
            nc.scalar.activation(sd2[:], var2[:], AF.Sqrt, bias=epst2[:])
            rstd2 = o2_pool.tile([1, D2], f32)
            nc.vector.reciprocal(rstd2[:], sd2[:])
            a2 = o2_pool.tile([1, D2], f32)
            nc.vector.tensor_mul(a2[:], rstd2[:], gb_s[:, 2 * D1 : 2 * D1 + D2])
            c2 = o2_pool.tile([1, D2], f32)
            nc.vector.tensor_mul(c2[:], mean2[:], a2[:])
            nc.vector.tensor_sub(c2[:], gb_s[:, 2 * D1 + D2 :], c2[:])

            # broadcast a2/c2 across partitions via ones-column matmul
            onesrow = o2_pool.tile([1, P], f32)
            nc.vector.memset(onesrow[:], 1.0)
            a2b = o2_pool.tile([P, D2], f32)
            c2b = o2_pool.tile([P, D2], f32)
            with tc.tile_pool(name="bn2p", bufs=2, space="PSUM") as bn2p:
                bps = bn2p.tile([P, D2], f32, tag="b2a")
                nc.tensor.matmul(out=bps[:], lhsT=onesrow[:], rhs=a2[:], start=True, stop=True)
                nc.vector.tensor_copy(out=a2b[:], in_=bps[:])
                cps = bn2p.tile([P, D2], f32, tag="b2c")
                nc.tensor.matmul(out=cps[:], lhsT=onesrow[:], rhs=c2[:], start=True, stop=True)
                nc.vector.tensor_copy(out=c2b[:], in_=cps[:])

            outst = o2_pool.tile([P, WPC, D2], f32)
            for w in range(WPC):
                nc.vector.tensor_mul(outst[:, w, :], o2_all[:, w, :], a2b[:])
                nc.vector.tensor_add(outst[:, w, :], outst[:, w, :], c2b[:])
            nc.gpsimd.dma_start(
                out=out_d[0:SLOTS, :].rearrange("(t p) d -> p t d", p=P),
                in_=outst[:],
            )

    return nc


# ---------------------------------------------------------------- entry point

def _run(x, edge_index, W1, gamma1, beta1, W2, gamma2, beta2, cfg, trace=False):
    from concourse.bass_utils import run_bass_kernel_spmd

    N = cfg["N"]
    pp = _preprocess(edge_index, N)
    xb, w1b, w2b = _pack_inputs(np.asarray(x, np.float32), np.asarray(W1, np.float32),
                                np.asarray(W2, np.float32), pp, cfg)
    nc = _build_kernel(cfg, pp, phases=int(__import__("os").environ.get("K_PHASES", "5")))
    nc.compile()

    shared = {
        "xb": np.ascontiguousarray(xb),
        "w1b": np.ascontiguousarray(w1b),
        "w2b": np.ascontiguousarray(w2b),
        "statmask": np.ascontiguousarray(pp["statmask"]),
        "gamma1": np.asarray(gamma1, np.float32).reshape(1, -1),
        "beta1": np.asarray(beta1, np.float32).reshape(1, -1),
        "gamma2": np.asarray(gamma2, np.float32).reshape(1, -1),
        "beta2": np.asarray(beta2, np.float32).reshape(1, -1),
    }
    in_maps = []
    for c in range(N_CORES):
        m = dict(shared)
        m["idx"] = np.ascontiguousarray(pp["idx_wrapped"][c])
        m["dismy"] = np.ascontiguousarray(pp["dismy"][c])
        in_maps.append(m)

    res = run_bass_kernel_spmd(nc, in_maps, core_ids=list(range(N_CORES)), trace=trace)
    _run.last_nc = nc

    D2 = cfg["D2"]
    out = np.empty((N, D2), np.float32)
    pos, core_of = pp["pos"], pp["core_of"]
    for c in range(N_CORES):
        nodes = np.flatnonzero(core_of == c)
        out[nodes] = res.results[c]["out"][pos[nodes]]
    _run.last_result = res
    return out


def kernel(x, edge_index, W1, b1, gamma1, beta1, W2, b2, gamma2, beta2):
    # b1/b2 cancel exactly through BatchNorm's mean subtraction; unused.
    return _run(x, edge_index, W1, gamma1, beta1, W2, gamma2, beta2, _FULL_CFG)
